# revision 1
# baseline (speedup 1.0000x reference)
"""Trainium2 Bass kernel for per-head attention.

Problem shapes: x [4, 1024, 12, 768]; per-head weights W_Q/K/V [12, 768, 64],
W_O [12, 64, 768]; the output projection keeps the head axis, so each of the
48 (batch, head) pairs is fully independent. Sharding: 6 pairs per core
across 8 NeuronCores (SPMD — same program, different per-core inputs).

Per-pair device pipeline (x_bh [S=1024, DM=768], S-tiles of 128):
  - host supplies xT [DM, S]; qT/kT/vT [64, S] come from matmuls with the
    128-row weight chunks as stationary operands (fp32r = TF32 rate).
  - scores are computed transposed (scoresT[k, q], k on partitions), causally
    chunked, so only the lower triangle is ever computed; softmax skips the
    max-subtraction (|scores| <~ 3; masked lanes use exp underflow semantics
    via a 0/1 mask multiply on the diagonal blocks).
  - a ones-column appended to v (v_aug [128, 65], built by PE-transposing
    vT tiles) makes the z-matmul also produce the softmax denominator
    (row 64 of zT_aug).
  - output projection uses Wo_aug whose row 64 is b_V @ W_O + b_O/H; dividing
    the projected result by the denominator (per-partition scalar, obtained
    by PE-transposing zT_aug s-tile slices) yields exactly
    softmax(scores) @ v @ W_O + b_V @ W_O + b_O/H.
"""

import numpy as np

import concourse.bacc as bacc
import concourse.mybir as mybir
from concourse.bass_utils import run_bass_kernel_spmd
from concourse.tile import TileContext

F32 = mybir.dt.float32
F32R = mybir.dt.float32r

B, S, H, DM, DH = 4, 1024, 12, 768, 64
N_CORES = 8
PAIRS_PER_CORE = (B * H) // N_CORES  # 6
MC = DM // 128  # m-chunks
ST = S // 128   # s-tiles
QC = S // 512   # q-chunks


def _build_kernel(n_pairs=PAIRS_PER_CORE):
    nc = bacc.Bacc()

    xT = nc.declare_dram_parameter("xT", [n_pairs, DM, S], F32R, isOutput=False)
    # one packed weight blob per pair (single DMA): columns are
    # [ wqk (MC*128, packed [Wk|Wq] chunks) | wv (MC*64) | wo_aug (768,
    #   rows 0:65) | bq (1) | bk (1) ]
    WQK0, WV0 = 0, MC * 128
    WO0 = WV0 + MC * DH
    BQ0 = WO0 + DM + 2  # wo_aug gains denom-indicator (WO0+DM) + pad cols
    BK0 = BQ0 + 1
    WBL = BK0 + 1
    wb = nc.declare_dram_parameter("wb", [n_pairs, 128, WBL], F32R, isOutput=False)
    ident = nc.declare_dram_parameter("ident", [128, 128], F32, isOutput=False)
    maskT = nc.declare_dram_parameter("maskT", [128, 128], F32R, isOutput=False)
    out = nc.declare_dram_parameter("out", [n_pairs, S, DM], F32, isOutput=True)

    with TileContext(nc) as tc:
        with (
            tc.tile_pool(name="const", bufs=1) as pconst,
            tc.tile_pool(name="xt", bufs=2) as px,
            tc.tile_pool(name="w", bufs=3) as pw,
            tc.tile_pool(name="qkv", bufs=2) as pqkv,
            tc.tile_pool(name="vaug", bufs=2) as pva,
            tc.tile_pool(name="exp", bufs=3) as pexp,
            tc.tile_pool(name="z", bufs=2) as pz,
            tc.tile_pool(name="rc", bufs=2) as prc,
            tc.tile_pool(name="outb", bufs=3) as pout,
            tc.tile_pool(name="ps_qkv", bufs=2, space="PSUM") as ppq,
            tc.tile_pool(name="ps_s", bufs=3, space="PSUM") as pps,
            tc.tile_pool(name="ps_z", bufs=1, space="PSUM") as ppz,
            # transposes + output-projection psums share two bank slots
            tc.tile_pool(name="ps_mix", bufs=2, space="PSUM") as ppmix,
        ):
            ident_t = pconst.tile([128, 128], F32, name="ident_t")
            nc.sync.dma_start(out=ident_t[:], in_=ident[:])
            mask_t = pconst.tile([128, 128], F32R, name="mask_t")
            nc.sync.dma_start(out=mask_t[:], in_=maskT[:])

            # PE warmup while the first x DMA is in flight: ~4us of dummy
            # matmuls flips the HAM clock gate to 8/8 (2.4 GHz) before the
            # real work starts.
            wscr = pconst.tile([128, 512], F32R, name="wscr")
            nc.vector.memset(wscr[:].bitcast(F32), 0.0)
            for wi in range(10):
                ps_w = pps.tile([128, 512], F32, name="ps_w", tag="ps_s")
                nc.tensor.matmul(ps_w[:], wscr[:, 0:128], wscr[:],
                                 start=True, stop=True)

            for p in range(n_pairs):
                wb_t = pw.tile([128, WBL], F32R, name="wb_t", tag="wb")
                nc.sync.dma_start(out=wb_t[:], in_=wb[p])
                wqk_t = wb_t[:, WQK0:WV0].rearrange("p (c d) -> p c d", d=128)
                wv_t = wb_t[:, WV0:WO0].rearrange("p (c d) -> p c d", d=DH)
                wo_t = wb_t[0:DH + 1, WO0:WO0 + DM + 2]
                bq_t = wb_t[:, BQ0:BQ0 + 1].bitcast(F32)
                bk_t = wb_t[0:DH, BK0:BK0 + 1].bitcast(F32)

                xta = px.tile([128, MC, S], F32R, name="xta", tag="xta")
                xTv = xT[p].rearrange("(c p) s -> p c s", p=128)
                if p == 0:
                    # fine-grained first load so the very first matmuls don't
                    # wait for the whole 3MB transfer
                    for mc in range(MC):
                        nc.sync.dma_start(
                            out=xta[:, mc, :], in_=xTv[:, mc, :])
                else:
                    nc.sync.dma_start(out=xta[:], in_=xTv)
                xt = [xta[:, mc, :] for mc in range(MC)]

                # QK projection, packed M=128 stationary -> [kT; qT] psum
                qT_sb = pqkv.tile([DH, S], F32R, name="qT_sb", tag="qT")
                kT_sb = pqkv.tile([DH, S], F32R, name="kT_sb", tag="kT")
                qstage = pqkv.tile([128, S], F32R, name="qstage", tag="qst")
                vT_sb = pqkv.tile([DH + 1, S], F32, name="vT_sb", tag="vT")
                nc.vector.memset(vT_sb[DH:DH + 1, :], 1.0)

                for sc in range(QC):
                    ps = ppq.tile([128, 512], F32, name="ps_qk", tag="ps_qkv")
                    for mc in range(MC):
                        nc.tensor.matmul(
                            ps[:], wqk_t[:, mc, :],
                            xt[mc][:, sc * 512:(sc + 1) * 512],
                            start=(mc == 0), stop=(mc == MC - 1))
                    cols = slice(sc * 512, (sc + 1) * 512)
                    # the two copies run on different engines so the qT
                    # shift DMA (which gates scores) isn't serialized behind
                    # the kT copy
                    nc.vector.tensor_scalar(
                        kT_sb[0:DH, cols], ps[0:DH, :], bk_t[:], None,
                        op0=mybir.AluOpType.add)
                    nc.scalar.activation(
                        qstage[DH:128, cols], ps[DH:128, :],
                        mybir.ActivationFunctionType.Identity,
                        bias=bq_t[DH:128, :], scale=1.0)
                    # partition shift 64:128 -> 0:64
                    nc.sync.dma_start(
                        out=qT_sb[0:DH, cols], in_=qstage[DH:128, cols])

                for sc in range(QC):
                    ps = ppq.tile([DH, 512], F32, name="ps_v", tag="ps_qkv")
                    for mc in range(MC):
                        nc.tensor.matmul(
                            ps[:], wv_t[:, mc, :],
                            xt[mc][:, sc * 512:(sc + 1) * 512],
                            start=(mc == 0), stop=(mc == MC - 1))
                    nc.vector.tensor_copy(
                        vT_sb[0:DH, sc * 512:(sc + 1) * 512], ps[:])

                # v_aug tiles [128, 65] via PE transpose
                v_aug = []
                for st in range(ST):
                    ps_t = ppmix.tile([128, DH + 1], F32, name="ps_vtr", tag="ps_mix")
                    nc.tensor.transpose(
                        ps_t[:], vT_sb[:, st * 128:(st + 1) * 128],
                        ident_t[0:DH + 1, 0:DH + 1])
                    va = pva.tile([128, DH + 1], F32R, name=f"va{st}", tag=f"va{st}")
                    nc.vector.tensor_copy(va[:], ps_t[:])
                    v_aug.append(va)

                # causal scoresT -> exp -> z accumulation.
                # The scores matmul runs two iterations ahead of the z matmul
                # so the PE never waits on the ACT exp in between.
                z_sb = pz.tile([DH + 1, S], F32R, name="z_sb", tag="z")
                for j in range(QC):
                    ps_z = ppz.tile([DH + 1, 512], F32, name="ps_z", tag="ps_z")
                    i_max = min(ST - 1, (512 * (j + 1) - 1) // 128)
                    pending = {}

                    def emit_scores(i, j=j):
                        c0 = max(128 * i, 512 * j)
                        L = 512 * (j + 1) - c0
                        ps_s = pps.tile([128, 512], F32, name="ps_s", tag="ps_s")
                        nc.tensor.matmul(
                            ps_s[:, 0:L], kT_sb[:, i * 128:(i + 1) * 128],
                            qT_sb[:, c0:c0 + L], start=True, stop=True)
                        pending[i] = (ps_s, c0, L)

                    emit_scores(0)
                    if i_max >= 1:
                        emit_scores(1)
                    for i in range(i_max + 1):
                        ps_s, c0, L = pending.pop(i)
                        ex = pexp.tile([128, 512], F32R, name="ex", tag="ex")
                        nc.scalar.activation(
                            ex[:, 0:L], ps_s[:, 0:L],
                            mybir.ActivationFunctionType.Exp,
                            bias=0.0, scale=0.125)
                        if c0 == 128 * i:
                            nc.vector.tensor_tensor(
                                ex[:, 0:128], ex[:, 0:128], mask_t[:],
                                op=mybir.AluOpType.mult)
                        if i + 2 <= i_max:
                            emit_scores(i + 2)
                        nc.tensor.matmul(
                            ps_z[:, c0 - 512 * j:512], v_aug[i][:], ex[:, 0:L],
                            start=(i == 0), stop=(i == i_max))
                    nc.vector.tensor_copy(z_sb[:, j * 512:(j + 1) * 512], ps_z[:])

                # output projection; wo_t's extra indicator column makes the
                # second matmul also emit the softmax denominator as a psum
                # column, so no PE transpose is needed for it
                obh = None
                for st in range(ST):
                    zsl = z_sb[:, st * 128:(st + 1) * 128]
                    g = st % 4
                    if g == 0:
                        obh = pout.tile([128, 4, DM], F32, name="obh", tag="obh")
                    ps_o2 = ppmix.tile([128, 512], F32, name="ps_o2", tag="ps_mix")
                    nc.tensor.matmul(
                        ps_o2[:, 0:DM - 512 + 2], zsl,
                        wo_t[:, 512:DM + 2], start=True, stop=True)
                    rc = prc.tile([128, 1], F32, name=f"rc{st}", tag=f"rc{st}")
                    nc.vector.reciprocal(rc[:], ps_o2[:, DM - 512:DM - 512 + 1])
                    ps_o1 = ppmix.tile([128, 512], F32, name="ps_o1", tag="ps_mix")
                    nc.tensor.matmul(
                        ps_o1[:, 0:512], zsl, wo_t[:, 0:512],
                        start=True, stop=True)
                    nc.scalar.mul(obh[:, g, 512:DM], ps_o2[:, 0:DM - 512], rc[:])
                    nc.vector.tensor_scalar(
                        obh[:, g, 0:512], ps_o1[:, 0:512], rc[:], None,
                        op0=mybir.AluOpType.mult)
                    if g == 3:
                        nc.gpsimd.dma_start(
                            out=out[p, (st - 3) * 128:(st + 1) * 128, :]
                            .rearrange("(g sp) m -> sp g m", sp=128),
                            in_=obh[:])

    nc.finalize()
    return nc


_NC_CACHE = {}


def _get_nc():
    if "nc" not in _NC_CACHE:
        _NC_CACHE["nc"] = _build_kernel()
    return _NC_CACHE["nc"]


def _make_pair_inputs(x, W_Q, b_Q, W_K, b_K, W_V, b_V, W_O, b_O, pairs):
    n = len(pairs)
    WQK0, WV0 = 0, MC * 128
    WO0 = WV0 + MC * DH
    BQ0 = WO0 + DM + 2  # wo_aug gains denom-indicator (WO0+DM) + pad cols
    BK0 = BQ0 + 1
    WBL = BK0 + 1
    m = {
        "xT": np.empty((n, DM, S), np.float32),
        "wb": np.zeros((n, 128, WBL), np.float32),
    }
    for idx, (b, h) in enumerate(pairs):
        m["xT"][idx] = x[b, :, h, :].T
        wb = m["wb"][idx]
        wqk = wb[:, WQK0:WV0].reshape(128, MC, 128)
        wqk[:, :, 0:DH] = W_K[h].reshape(MC, 128, DH).transpose(1, 0, 2)
        wqk[:, :, DH:128] = W_Q[h].reshape(MC, 128, DH).transpose(1, 0, 2)
        wb[:, WV0:WO0].reshape(128, MC, DH)[:] = \
            W_V[h].reshape(MC, 128, DH).transpose(1, 0, 2)
        wb[0:DH, WO0:WO0 + DM] = W_O[h]
        wb[DH, WO0:WO0 + DM] = b_V[h] @ W_O[h] + b_O / H
        wb[DH, WO0 + DM] = 1.0
        wb[DH:128, BQ0] = b_Q[h]
        wb[0:DH, BK0] = b_K[h]
    m["ident"] = np.eye(128, dtype=np.float32)
    ql = np.arange(128)
    m["maskT"] = (ql[None, :] >= ql[:, None]).astype(np.float32)
    return m


def kernel(normalized_resid_pre, W_Q, b_Q, W_K, b_K, W_V, b_V, W_O, b_O):
    x = np.ascontiguousarray(np.asarray(normalized_resid_pre, dtype=np.float32))
    args = tuple(np.asarray(a, dtype=np.float32)
                 for a in (W_Q, b_Q, W_K, b_K, W_V, b_V, W_O, b_O))

    pairs = [(b, h) for b in range(B) for h in range(H)]
    nc = _get_nc()
    in_maps = [
        _make_pair_inputs(x, *args, pairs[c * PAIRS_PER_CORE:(c + 1) * PAIRS_PER_CORE])
        for c in range(N_CORES)
    ]
    res = run_bass_kernel_spmd(nc, in_maps, list(range(N_CORES)))

    got = np.empty((B, S, H, DM), np.float32)
    for c in range(N_CORES):
        for u in range(PAIRS_PER_CORE):
            b, h = pairs[c * PAIRS_PER_CORE + u]
            got[b, :, h, :] = res.results[c]["out"][u]
    return got



# revision 2
# speedup vs baseline: 1.3850x; 1.3850x over previous
"""Trainium2 Bass kernel for per-head attention (fp16 v2).

Problem shapes: x [4, 1024, 12, 768]; per-head weights W_Q/K/V [12, 768, 64],
W_O [12, 64, 768]; the output projection keeps the head axis, so each of the
48 (batch, head) pairs is fully independent. Sharding: 6 pairs per core
across 8 NeuronCores (SPMD), grouped so each core sees only 2 distinct heads
(one head x 4 batches + one half-head x 2 batches) -> 2 weight DMAs per core.

All DMA-side tensors are fp16 (halves HBM traffic vs fp32; PE runs fp16 at
full rate with FWL fast weight loads; DVE gets 2x modes). PSUM stays fp32.

Per-pair device pipeline (x_bh [S=1024, DM=768], S-tiles of 128):
  - host supplies xT [DM, S] fp16; [kT|qT] come from matmuls with packed
    128-col weight chunks stationary; biases applied on DVE/ACT during the
    psum->SBUF copy; partition-shift DMAs give each of q/k at both SBUF
    partition halves (0:64 and 64:128) for row-tiled score matmuls.
  - v is computed directly in [s, d] layout using the xT chunks as the
    stationary operand (out = xT_chunk.T @ W_V_chunk), so no PE transposes
    are needed; a ones-column appended to v makes the z-matmul also emit the
    softmax denominator (row 64 of zT_aug).
  - scores are computed transposed (scoresT[k, q]) causally chunked, two key
    blocks at a time via row tiling (block i on array rows 0:64, block i+1 on
    rows 64:128, concurrent); softmax skips the max-subtraction (|scores|
    <~ 3); masked lanes use a 0/1 mask multiply on the diagonal blocks.
  - output projection uses Wo_aug whose row 64 is b_V @ W_O + b_O/H and an
    indicator column that makes the projection also emit the softmax
    denominator; dividing by it during psum evacuation (split DVE/ACT)
    yields softmax(scores) @ v @ W_O + b_V @ W_O + b_O/H exactly.
"""

import numpy as np

import concourse.bacc as bacc
import concourse.mybir as mybir
from concourse.bass_utils import run_bass_kernel_spmd
from concourse.tile import TileContext

F16 = mybir.dt.float16
F32 = mybir.dt.float32

B, S, H, DM, DH = 4, 1024, 12, 768, 64
N_CORES = 8
PAIRS_PER_CORE = (B * H) // N_CORES  # 6
MC = DM // 128  # m-chunks
ST = S // 128   # s-tiles
QC = S // 512   # q-chunks

# packed per-head weight blob (fp16 columns):
# [ wqk (MC*128, per-chunk [Wk|Wq]) | wv (MC*64, moving-layout W_V chunks) |
#   wo_aug (rows 0:65, 770 cols)    | bq (f32, 2 cols) | bk (f32, 2 cols) ]
WQK0, WV0 = 0, MC * 128
WO0 = WV0 + MC * DH
BQ0 = WO0 + DM + 2
BK0 = BQ0 + 2
WBL = BK0 + 2


def _build_kernel(n_pairs=PAIRS_PER_CORE):
    nc = bacc.Bacc()

    xT = nc.declare_dram_parameter("xT", [n_pairs, DM, S], F16, isOutput=False)
    wb = nc.declare_dram_parameter("wb", [2, 128, WBL], F16, isOutput=False)
    maskT = nc.declare_dram_parameter("maskT", [128, 128], F16, isOutput=False)
    out = nc.declare_dram_parameter("out", [n_pairs, S, DM], F16, isOutput=True)

    with TileContext(nc) as tc:
        with (
            tc.tile_pool(name="const", bufs=1) as pconst,
            tc.tile_pool(name="xt", bufs=2) as px,
            tc.tile_pool(name="w", bufs=2) as pw,
            tc.tile_pool(name="qkv", bufs=2) as pqkv,
            tc.tile_pool(name="vaug", bufs=2) as pva,
            tc.tile_pool(name="exp", bufs=3) as pexp,
            tc.tile_pool(name="z", bufs=2) as pz,
            tc.tile_pool(name="rc", bufs=2) as prc,
            tc.tile_pool(name="outb", bufs=3) as pout,
            tc.tile_pool(name="ps_qkv", bufs=2, space="PSUM") as ppq,
            tc.tile_pool(name="ps_s", bufs=3, space="PSUM") as pps,
            tc.tile_pool(name="ps_z", bufs=1, space="PSUM") as ppz,
            tc.tile_pool(name="ps_o", bufs=2, space="PSUM") as ppo,
        ):
            mask_t = pconst.tile([128, 128], F16, name="mask_t")
            nc.sync.dma_start(out=mask_t[:], in_=maskT[:])

            # PE warmup while the first x DMA is in flight: ~4us of dummy
            # matmuls flips the HAM clock gate to 8/8 (2.4 GHz) before the
            # real work starts.
            wscr = pconst.tile([128, 512], F16, name="wscr")
            nc.vector.memset(wscr[:], 0.0)
            for wi in range(10):
                ps_w = pps.tile([128, 512], F32, name="ps_w", tag="ps_s")
                nc.tensor.matmul(ps_w[:], wscr[:, 0:128], wscr[:],
                                 start=True, stop=True)

            p = 0
            for g, gsize in ((0, 4), (1, 2)):
                wb_t = pw.tile([128, WBL], F16, name="wb_t", tag="wb")
                nc.sync.dma_start(out=wb_t[:], in_=wb[g])
                wqk_t = wb_t[:, WQK0:WV0].rearrange("p (c d) -> p c d", d=128)
                wv_t = wb_t[:, WV0:WO0].rearrange("p (c d) -> p c d", d=DH)
                wo_t = wb_t[0:DH + 1, WO0:WO0 + DM + 2]
                bq_t = wb_t[DH:128, BQ0:BQ0 + 2].bitcast(F32)
                bk_t = wb_t[0:DH, BK0:BK0 + 2].bitcast(F32)

                for _u in range(gsize):
                    xta = px.tile([128, MC, S], F16, name="xta", tag="xta")
                    xTv = xT[p].rearrange("(c p) s -> p c s", p=128)
                    if p == 0:
                        # fine-grained first load so the very first matmuls
                        # don't wait for the whole transfer
                        for mc in range(MC):
                            nc.sync.dma_start(
                                out=xta[:, mc, :], in_=xTv[:, mc, :])
                    else:
                        nc.sync.dma_start(out=xta[:], in_=xTv)

                    # QK projection, packed M=128 stationary -> [kT; qT]
                    # psum; partition-shift DMAs replicate each half so both
                    # q and k exist at partitions 0:64 (lo) and 64:128 (hi).
                    qhi = pqkv.tile([128, S], F16, name="qhi", tag="qhi")
                    qlo = pqkv.tile([DH, S], F16, name="qlo", tag="qlo")
                    klo = pqkv.tile([DH, S], F16, name="klo", tag="klo")
                    khi = pqkv.tile([128, S], F16, name="khi", tag="khi")
                    for sc in range(QC):
                        ps = ppq.tile([128, 512], F32, name="ps_qk",
                                      tag="ps_qkv")
                        for mc in range(MC):
                            nc.tensor.matmul(
                                ps[:], wqk_t[:, mc, :],
                                xta[:, mc, sc * 512:(sc + 1) * 512],
                                start=(mc == 0), stop=(mc == MC - 1))
                        cols = slice(sc * 512, (sc + 1) * 512)
                        nc.vector.tensor_scalar(
                            klo[:, cols], ps[0:DH, :], bk_t[:], None,
                            op0=mybir.AluOpType.add)
                        nc.scalar.activation(
                            qhi[DH:128, cols], ps[DH:128, :],
                            mybir.ActivationFunctionType.Identity,
                            bias=bq_t[:], scale=1.0)
                        nc.sync.dma_start(
                            out=khi[DH:128, cols], in_=klo[:, cols])
                        nc.sync.dma_start(
                            out=qlo[:, cols], in_=qhi[DH:128, cols])

                    # V projection with the xT chunks stationary: psum gets
                    # v directly in [s, d] layout (8 groups of 64 cols in
                    # one bank), no PE transpose needed.
                    ps_v = ppq.tile([128, 512], F32, name="ps_v", tag="ps_qkv")
                    for st in range(ST):
                        for mc in range(MC):
                            nc.tensor.matmul(
                                ps_v[:, st * DH:(st + 1) * DH],
                                xta[:, mc, st * 128:(st + 1) * 128],
                                wv_t[:, mc, :],
                                start=(mc == 0), stop=(mc == MC - 1))
                    v_aug = []
                    for st in range(ST):
                        va = pva.tile([128, DH + 1], F16, name=f"va{st}",
                                      tag=f"va{st}")
                        nc.vector.memset(va[:, DH:DH + 1], 1.0)
                        nc.vector.tensor_copy(
                            va[:, 0:DH], ps_v[:, st * DH:(st + 1) * DH])
                        v_aug.append(va)

                    # causal scoresT -> exp -> z accumulation. Score matmuls
                    # run in row-tiled pairs: block 2a on array rows 0:64,
                    # block 2a+1 on rows 64:128, issued back-to-back so they
                    # execute concurrently in the PE array.
                    z_t = pz.tile([DH + 1, S], F16, name="z_t", tag="z")
                    for j in range(QC):
                        ps_zb = ppz.tile([DH + 1, 512], F32, name="ps_z",
                                         tag="ps_z")
                        i_max = min(ST - 1, (512 * (j + 1) - 1) // 128)
                        npair = (i_max + 1) // 2
                        pend = {}

                        def emit_pair(a, j=j, pend=pend):
                            for t in (0, 1):
                                i = 2 * a + t
                                c0 = max(128 * i, 512 * j)
                                L = 512 * (j + 1) - c0
                                ps_s = pps.tile([128, 512], F32, name="ps_s",
                                                tag="ps_s")
                                if t == 0:
                                    kst = klo[:, i * 128:(i + 1) * 128]
                                    qmv = qlo[:, c0:c0 + L]
                                else:
                                    kst = khi[DH:128, i * 128:(i + 1) * 128]
                                    qmv = qhi[DH:128, c0:c0 + L]
                                nc.tensor.matmul(ps_s[:, 0:L], kst, qmv,
                                                 start=True, stop=True)
                                pend[i] = (ps_s, c0, L)

                        emit_pair(0)
                        if npair > 1:
                            emit_pair(1)
                        for i in range(i_max + 1):
                            ps_s, c0, L = pend.pop(i)
                            ex = pexp.tile([128, 512], F16, name="ex",
                                           tag="ex")
                            nc.scalar.activation(
                                ex[:, 0:L], ps_s[:, 0:L],
                                mybir.ActivationFunctionType.Exp,
                                bias=0.0, scale=0.125)
                            if c0 == 128 * i:
                                nc.vector.tensor_tensor(
                                    ex[:, 0:128], ex[:, 0:128], mask_t[:],
                                    op=mybir.AluOpType.mult)
                            if i % 2 == 1 and i // 2 + 2 < npair:
                                emit_pair(i // 2 + 2)
                            nc.tensor.matmul(
                                ps_zb[:, c0 - 512 * j:512], v_aug[i][:],
                                ex[:, 0:L],
                                start=(i == 0), stop=(i == i_max))
                        nc.vector.tensor_copy(
                            z_t[:, j * 512:(j + 1) * 512], ps_zb[:])

                    # output projection; wo_t's indicator column makes the
                    # second matmul also emit the softmax denominator as a
                    # psum column
                    obh = None
                    for st in range(ST):
                        zsl = z_t[:, st * 128:(st + 1) * 128]
                        gg = st % 4
                        if gg == 0:
                            obh = pout.tile([128, 4, DM], F16, name="obh",
                                            tag="obh")
                        ps_o2 = ppo.tile([128, 512], F32, name="ps_o2",
                                         tag="ps_o")
                        nc.tensor.matmul(
                            ps_o2[:, 0:DM - 512 + 2], zsl,
                            wo_t[:, 512:DM + 2], start=True, stop=True)
                        rc = prc.tile([128, 1], F32, name=f"rc{st}",
                                      tag=f"rc{st}")
                        nc.vector.reciprocal(
                            rc[:], ps_o2[:, DM - 512:DM - 512 + 1])
                        ps_o1 = ppo.tile([128, 512], F32, name="ps_o1",
                                         tag="ps_o")
                        nc.tensor.matmul(
                            ps_o1[:, 0:512], zsl, wo_t[:, 0:512],
                            start=True, stop=True)
                        nc.scalar.mul(
                            obh[:, gg, 512:DM], ps_o2[:, 0:DM - 512], rc[:])
                        nc.vector.tensor_scalar(
                            obh[:, gg, 0:512], ps_o1[:, 0:512], rc[:], None,
                            op0=mybir.AluOpType.mult)
                        if gg == 3:
                            nc.gpsimd.dma_start(
                                out=out[p, (st - 3) * 128:(st + 1) * 128, :]
                                .rearrange("(g sp) m -> sp g m", sp=128),
                                in_=obh[:])
                    p += 1

    nc.finalize()
    return nc


_NC_CACHE = {}


def _get_nc():
    if "nc" not in _NC_CACHE:
        _NC_CACHE["nc"] = _build_kernel()
    return _NC_CACHE["nc"]


def _core_pairs(c):
    """6 (batch, head) pairs for core c: head c x batches 0..3, plus half of
    head 8 + c//2 (2 batches)."""
    pairs = [(b, c) for b in range(B)]
    h2 = 8 + c // 2
    b0 = (c % 2) * 2
    pairs += [(b0, h2), (b0 + 1, h2)]
    return pairs


def _head_blob(W_Q, b_Q, W_K, b_K, W_V, b_V, W_O, b_O, h):
    wbh = np.zeros((128, WBL), np.float16)
    wqk = wbh[:, WQK0:WV0].reshape(128, MC, 128)
    wqk[:, :, 0:DH] = W_K[h].reshape(MC, 128, DH).transpose(1, 0, 2)
    wqk[:, :, DH:128] = W_Q[h].reshape(MC, 128, DH).transpose(1, 0, 2)
    wbh[:, WV0:WO0].reshape(128, MC, DH)[:] = \
        W_V[h].reshape(MC, 128, DH).transpose(1, 0, 2)
    wbh[0:DH, WO0:WO0 + DM] = W_O[h]
    wbh[DH, WO0:WO0 + DM] = b_V[h] @ W_O[h] + b_O / H
    wbh[DH, WO0 + DM] = 1.0
    wbh[DH:128, BQ0:BQ0 + 2] = \
        np.ascontiguousarray(b_Q[h], np.float32).view(np.float16).reshape(DH, 2)
    wbh[0:DH, BK0:BK0 + 2] = \
        np.ascontiguousarray(b_K[h], np.float32).view(np.float16).reshape(DH, 2)
    return wbh


def _make_core_inputs(x, W_Q, b_Q, W_K, b_K, W_V, b_V, W_O, b_O, c):
    pairs = _core_pairs(c)
    m = {
        "xT": np.empty((PAIRS_PER_CORE, DM, S), np.float16),
        "wb": np.empty((2, 128, WBL), np.float16),
    }
    for idx, (b, h) in enumerate(pairs):
        m["xT"][idx] = x[b, :, h, :].T
    args = (W_Q, b_Q, W_K, b_K, W_V, b_V, W_O, b_O)
    m["wb"][0] = _head_blob(*args, pairs[0][1])
    m["wb"][1] = _head_blob(*args, pairs[4][1])
    ql = np.arange(128)
    m["maskT"] = (ql[None, :] >= ql[:, None]).astype(np.float16)
    return m


def kernel(normalized_resid_pre, W_Q, b_Q, W_K, b_K, W_V, b_V, W_O, b_O):
    x = np.ascontiguousarray(np.asarray(normalized_resid_pre, dtype=np.float32))
    args = tuple(np.asarray(a, dtype=np.float32)
                 for a in (W_Q, b_Q, W_K, b_K, W_V, b_V, W_O, b_O))

    nc = _get_nc()
    in_maps = [_make_core_inputs(x, *args, c) for c in range(N_CORES)]
    res = run_bass_kernel_spmd(nc, in_maps, list(range(N_CORES)))

    got = np.empty((B, S, H, DM), np.float32)
    for c in range(N_CORES):
        ro = np.asarray(res.results[c]["out"], np.float32)
        for idx, (b, h) in enumerate(_core_pairs(c)):
            got[b, :, h, :] = ro[idx]
    return got
